# revision 13
# baseline (speedup 1.0000x reference)
"""Distributed Trainium2 kernel for nn_Curating_of_attention_mask.

Math: batch (3,1280,1280) -> 6400 patches of 16x16 -> per-patch 3x3 channel
gram -> pairwise squared-distance matrix (6400,6400) -> global min/max
normalize -> (1,6400,6400).

Key simplification: the final (d - min)/(max - min) normalization is
invariant to any positive affine rescaling of d, so the reference's /768
and /9 factors are dropped entirely.  With the 6 unique gram entries
m = [m00,m01,m02,m11,m12,m22] (raw dot products) and
q = m00^2+2*m01^2+2*m02^2+m11^2+2*m12^2+m22^2, define per patch
  v = [m00,m01,m02,m11,m12,m22, 1, q]          (column features)
  u = [-2*m00,-4*m01,-4*m02,-2*m11,-4*m12,-2*m22, q, 1]   (row features)
then raw[i,j] = u_i . v_j = q_i + q_j - 2*<gram_i, gram_j>, and
out = (raw - min) / (max - min) = raw*s + t with s = 1/(M-m), t = -m*s.

Sharding: patch dimension across 8 cores (core k owns image rows
[160k,160k+160) = patch rows [10k,10k+10) = patches [800k, 800k+800)).
Each core computes u/v for its 800 patches, AllGathers v (8x800 floats),
computes its [800, 6400] slice of raw twice (pass 1 reduces min/max, one
AllReduce(max) of [max,-min], pass 2 applies the affine and writes out).
"""

import numpy as np

import concourse.bass as bass
import concourse.bass_isa as bass_isa
import concourse.mybir as mybir
import concourse.tile as tile
from concourse.bass_utils import run_bass_kernel_spmd

F32 = mybir.dt.float32
F32R = mybir.dt.float32r
AX = mybir.AluOpType
AFT = mybir.ActivationFunctionType

N_CORES = 8
C, H, W = 3, 1280, 1280
PS = 16
HP, WP = H // PS, W // PS            # 80, 80
N = HP * WP                          # 6400
H_LOC = H // N_CORES                 # 160 image rows per core
TP = H_LOC // PS                     # 10 patch-rows per core
N_LOC = TP * WP                      # 800 patches per core
K = 8                                # feature dim of u/v

# unique gram entries (channel pairs); columns 0..5 of v
PAIRS = [(0, 0), (0, 1), (0, 2), (1, 1), (1, 2), (2, 2)]
NEG_BIG = -3.0e38

# Distances are invariant to subtracting a constant vector from every
# patch's gram, so center by the expected gram of unit-variance noise
# (E[sum_e x_e^2] = 256 on the diagonal).  This shrinks feature magnitudes
# ~30x, which shrinks every rounding error by the same factor.
DIAG_COLS = (0, 3, 5)
CENTER = 256.0

# "f32r" = float32r matmuls (1 cyc/row, inputs rounded to ~TF32 precision)
# "f32"  = plain float32 matmuls (4 cyc/row, exact)
MM_MODE = "f32r"

# output tiling
M_TILES = [(ms, min(128, N_LOC - ms)) for ms in range(0, N_LOC, 128)]   # 7
N_TILES = [(ns, min(512, N - ns)) for ns in range(0, N, 512)]           # 13
NTOT = len(M_TILES) * len(N_TILES)                                      # 91

# walrus in this container accepts at most 1 sync-wait command per
# instruction; Tile's tail drain can carry several.  Split extras onto
# preceding NOPs on the same engine (stream order preserves semantics).
_MAX_WAITS = 1


def _split_sync_waits(nc):
    n_fixed = 0
    for func in nc.m.functions:
        for bb in func.blocks:
            new_insts = []
            for inst in bb.instructions:
                si = inst.sync_info
                if si is not None and si.on_wait and len(si.on_wait) > _MAX_WAITS:
                    waits = list(si.on_wait)
                    keep = waits[-_MAX_WAITS:]
                    extra = waits[:-_MAX_WAITS]
                    chunks = [
                        extra[i : i + _MAX_WAITS]
                        for i in range(0, len(extra), _MAX_WAITS)
                    ]
                    for ci, chunk in enumerate(chunks):
                        nop = mybir.InstNoOp(
                            name=f"{inst.name}-waitsplit-{ci}",
                            engine=inst.engine,
                            ins=[],
                            outs=[],
                            sync_info=mybir.SyncInfo(on_wait=chunk, on_update=[]),
                        )
                        new_insts.append(nop)
                        n_fixed += 1
                    si.on_wait = keep
                new_insts.append(inst)
            bb.instructions[:] = new_insts
    return n_fixed


def _build():
    nc = bass.Bass(num_devices=N_CORES)
    x = nc.dram_tensor("x", [C, H_LOC, W], F32, kind="ExternalInput")
    out = nc.dram_tensor("out", [N_LOC, N], F32, kind="ExternalOutput")
    groups = [list(range(N_CORES))]

    with tile.TileContext(nc, num_cores=N_CORES) as tc:
        with (
            tc.tile_pool(name="dram", bufs=1, space="DRAM") as dpool,
            tc.tile_pool(name="cst", bufs=1) as cst,
            tc.tile_pool(name="xin", bufs=2) as xin,
            tc.tile_pool(name="gram", bufs=2) as gpool,
            tc.tile_pool(name="big", bufs=1) as big,
            tc.tile_pool(name="scr", bufs=4) as scrp,
            tc.tile_pool(name="osb", bufs=4) as osbp,
            tc.tile_pool(name="ps", bufs=6, space="PSUM") as psp,
        ):
            v_dram = dpool.tile([K, N_LOC], F32, name="v_dram")
            u_dram = dpool.tile([K, N_LOC], F32, name="u_dram")
            vall = dpool.tile([K * N_CORES, N_LOC], F32, addr_space="Shared",
                              name="vall")
            cc_in = dpool.tile([1, 8], F32, name="cc_in")
            cc_out = dpool.tile([1, 8], F32, addr_space="Shared", name="cc_out")

            # ---- phase A: per-patch gram features ----
            for t in range(TP):
                xs = []
                for c in range(C):
                    xt = xin.tile([WP, PS * PS], F32, name=f"xt{c}", tag=f"xt{c}")
                    nc.sync.dma_start(
                        xt.rearrange("w (a b) -> w a b", b=PS),
                        x[c, PS * t : PS * (t + 1), :].rearrange(
                            "a (w b) -> w a b", b=PS
                        ),
                    )
                    xs.append(xt)
                vt = gpool.tile([WP, K], F32, name="vt", tag="vt")
                ut = gpool.tile([WP, K], F32, name="ut", tag="ut")
                wt = gpool.tile([WP, 6], F32, name="wt", tag="wt")
                prod = gpool.tile([WP, PS * PS], F32, name="prod", tag="prod")
                scr2 = gpool.tile([WP, PS * PS], F32, name="scr2", tag="scr2")
                # diagonal entries on ACT (square + free running sum)
                for col, (a, b) in enumerate(PAIRS):
                    if a == b:
                        nc.scalar.activation(
                            scr2[:, :], xs[a][:, :], AFT.Square,
                            accum_out=vt[:, col : col + 1],
                        )
                # off-diagonal: DVE product, ACT copy with running sum
                for col, (a, b) in enumerate(PAIRS):
                    if a != b:
                        nc.vector.tensor_mul(prod[:, :], xs[a][:, :], xs[b][:, :])
                        nc.scalar.activation(
                            scr2[:, :], prod[:, :], AFT.Copy,
                            accum_out=vt[:, col : col + 1],
                        )
                # center the diagonal gram entries
                for dc in DIAG_COLS:
                    nc.vector.tensor_scalar_add(
                        vt[:, dc : dc + 1], vt[:, dc : dc + 1], -CENTER
                    )
                # wt = m with off-diagonals doubled
                nc.scalar.copy(wt[:, :], vt[:, 0:6])
                nc.scalar.mul(wt[:, 1:3], wt[:, 1:3], 2.0)
                nc.scalar.mul(wt[:, 4:5], wt[:, 4:5], 2.0)
                # q = sum(m * w)
                prod6 = gpool.tile([WP, 6], F32, name="prod6", tag="prod6")
                nc.vector.tensor_mul(prod6[:, :], vt[:, 0:6], wt[:, :])
                nc.vector.tensor_reduce(
                    out=vt[:, 7:8], in_=prod6[:, :],
                    axis=mybir.AxisListType.X, op=AX.add,
                )
                # u cols 0..5 = -2 * w
                nc.scalar.activation(ut[:, 0:6], wt[:, :], AFT.Copy, scale=-2.0)
                nc.scalar.copy(ut[:, 6:7], vt[:, 7:8])
                nc.vector.memset(vt[:, 6:7], 1.0)
                nc.vector.memset(ut[:, 7:8], 1.0)
                # transpose-store [80, 8] -> [8, 800] slice
                nc.sync.dma_start(
                    v_dram[:, WP * t : WP * (t + 1)].rearrange("f w -> w f"),
                    vt[:, :],
                )
                nc.sync.dma_start(
                    u_dram[:, WP * t : WP * (t + 1)].rearrange("f w -> w f"),
                    ut[:, :],
                )

            # ---- all-gather v across cores ----
            nc.gpsimd.collective_compute(
                "AllGather",
                AX.bypass,
                replica_groups=groups,
                ins=[v_dram.opt()],
                outs=[vall.opt()],
            )

            rhs = big.tile([K, N], F32, name="rhs")
            nc.sync.dma_start(
                rhs.rearrange("f (r l) -> f r l", l=N_LOC),
                vall.rearrange("(r f) l -> f r l", f=K),
            )
            lhsT = big.tile([K, N_LOC], F32, name="lhsT")
            nc.sync.dma_start(lhsT[:, :], u_dram[:, :])

            if MM_MODE == "f32r":
                # f32r matmul inputs must be produced pre-rounded
                mm_rhs = big.tile([K, N], F32R, name="mm_rhs")
                nc.vector.tensor_copy(mm_rhs[:, :], rhs[:, :])
                mm_lhsT = big.tile([K, N_LOC], F32R, name="mm_lhsT")
                nc.vector.tensor_copy(mm_lhsT[:, :], lhsT[:, :])
            else:
                mm_rhs, mm_lhsT = rhs, lhsT

            # ---- pass 1: min/max of raw ----
            racc = cst.tile([128, 2 * NTOT], F32, name="racc")
            nc.vector.memset(racc[:, :], NEG_BIG)
            idx = 0
            for ms, mh in M_TILES:
                for ns, nw in N_TILES:
                    ps_t = psp.tile([128, 512], F32, name="ps1", tag="ps")
                    nc.tensor.matmul(
                        ps_t[0:mh, 0:nw],
                        lhsT=mm_lhsT[:, ms : ms + mh],
                        rhs=mm_rhs[:, ns : ns + nw],
                        start=True,
                        stop=True,
                    )
                    sc = scrp.tile([128, 512], F32, name="sc", tag="sc")
                    nc.scalar.activation(sc[0:mh, 0:nw], ps_t[0:mh, 0:nw], AFT.Copy)
                    nc.vector.tensor_reduce(
                        out=racc[0:mh, idx : idx + 1],
                        in_=sc[0:mh, 0:nw],
                        axis=mybir.AxisListType.X,
                        op=AX.max,
                    )
                    nc.vector.tensor_reduce(
                        out=racc[0:mh, NTOT + idx : NTOT + idx + 1],
                        in_=sc[0:mh, 0:nw],
                        axis=mybir.AxisListType.X,
                        op=AX.min,
                        negate=True,
                    )
                    idx += 1

            # fold 91 tiles -> [128,2] = [max, -min], then partitions, then cores
            gpair = cst.tile([128, 2], F32, name="gpair")
            nc.vector.tensor_reduce(
                out=gpair[:, :],
                in_=racc.rearrange("p (h n) -> p h n", h=2),
                axis=mybir.AxisListType.X,
                op=AX.max,
            )
            # cross-partition reduce: bounce [128,2] through DRAM transposed
            gp_dram = dpool.tile([128, 2], F32, name="gp_dram")
            nc.sync.dma_start(gp_dram[:, :], gpair[:, :])
            gpT = cst.tile([2, 128], F32, name="gpT")
            nc.sync.dma_start(gpT[:, :], gp_dram.rearrange("w f -> f w"))
            gg = cst.tile([2, 1], F32, name="gg")
            nc.vector.tensor_reduce(
                out=gg[:, :], in_=gpT[:, :], axis=mybir.AxisListType.X, op=AX.max
            )
            sc8 = cst.tile([1, 8], F32, name="sc8")
            nc.vector.memset(sc8[:, :], NEG_BIG)
            nc.sync.dma_start(cc_in[:, :], sc8[:, :])
            nc.sync.dma_start(
                cc_in[0:1, 0:2].rearrange("a b -> b a"), gg[:, :]
            )
            nc.gpsimd.collective_compute(
                "AllReduce",
                AX.max,
                replica_groups=groups,
                ins=[cc_in.opt()],
                outs=[cc_out.opt()],
            )
            mm_sb = cst.tile([1, 2], F32, name="mm_sb")
            nc.sync.dma_start(mm_sb[:, :], cc_out[0:1, 0:2])
            # broadcast [1,2] -> [128,2] via ones-matmul into PSUM
            ones_col = cst.tile([1, 128], F32, name="ones_col")
            nc.vector.memset(ones_col[:, :], 1.0)
            ps_bc = psp.tile([128, 2], F32, name="ps_bc", tag="ps_bc", bufs=1)
            nc.tensor.matmul(
                ps_bc[:, :],
                lhsT=ones_col[0:1, :],
                rhs=mm_sb[0:1, :],
                start=True,
                stop=True,
            )
            # s = 1/(M - m); t = -m * s   (bc_sb = [M, -m] per partition)
            bc_sb = cst.tile([128, 2], F32, name="bc_sb")
            nc.vector.tensor_copy(bc_sb[:, :], ps_bc[:, :])
            rng = cst.tile([128, 1], F32, name="rng")
            nc.vector.tensor_add(rng[:, :], bc_sb[:, 0:1], bc_sb[:, 1:2])
            s_sb = cst.tile([128, 1], F32, name="s_sb")
            nc.vector.reciprocal(s_sb[:, :], rng[:, :])
            t_sb = cst.tile([128, 1], F32, name="t_sb")
            nc.vector.tensor_mul(t_sb[:, :], bc_sb[:, 1:2], s_sb[:, :])

            # ---- pass 2: recompute, normalize, write out ----
            k2 = 0
            for ms, mh in M_TILES:
                for ns, nw in N_TILES:
                    ps_t = psp.tile([128, 512], F32, name="ps2", tag="ps")
                    nc.tensor.matmul(
                        ps_t[0:mh, 0:nw],
                        lhsT=mm_lhsT[:, ms : ms + mh],
                        rhs=mm_rhs[:, ns : ns + nw],
                        start=True,
                        stop=True,
                    )
                    ob = osbp.tile([128, 512], F32, name="ob", tag="ob")
                    if k2 % 2 == 0:
                        nc.vector.tensor_scalar(
                            out=ob[0:mh, 0:nw],
                            in0=ps_t[0:mh, 0:nw],
                            scalar1=s_sb[0:mh, 0:1],
                            scalar2=t_sb[0:mh, 0:1],
                            op0=AX.mult,
                            op1=AX.add,
                        )
                    else:
                        nc.scalar.activation(
                            ob[0:mh, 0:nw],
                            ps_t[0:mh, 0:nw],
                            AFT.Identity,
                            bias=t_sb[0:mh, 0:1],
                            scale=s_sb[0:mh, 0:1],
                        )
                    nc.sync.dma_start(out[ms : ms + mh, ns : ns + nw], ob[0:mh, 0:nw])
                    k2 += 1

    _split_sync_waits(nc)
    return nc


_NC_CACHE = []


def kernel(batch: np.ndarray) -> np.ndarray:
    batch = np.asarray(batch, dtype=np.float32)
    assert batch.shape == (C, H, W)
    if not _NC_CACHE:
        _NC_CACHE.append(_build())
    nc = _NC_CACHE[0]
    in_maps = [
        {"x": np.ascontiguousarray(batch[:, k * H_LOC : (k + 1) * H_LOC, :])}
        for k in range(N_CORES)
    ]
    res = run_bass_kernel_spmd(nc, in_maps, core_ids=list(range(N_CORES)))
    full = np.concatenate([res.results[k]["out"] for k in range(N_CORES)], axis=0)
    return full[None].astype(np.float32)


# revision 17
# speedup vs baseline: 1.3769x; 1.3769x over previous
"""Distributed Trainium2 kernel for nn_Curating_of_attention_mask.

Math: batch (3,1280,1280) -> 6400 patches of 16x16 -> per-patch 3x3 channel
gram -> pairwise squared-distance matrix (6400,6400) -> global min/max
normalize -> (1,6400,6400).

Key simplifications:
 - (d - min)/(max - min) is invariant to positive affine rescaling of d, so
   the reference's /768 and /9 factors are dropped.
 - Distances are invariant to subtracting a constant vector from every
   patch's gram, so grams are centered by the expected gram of unit-variance
   noise (256 on the diagonal); this shrinks magnitudes ~30x and with it
   every rounding error.
 - With the 6 unique (centered) gram entries m = [m00,m11,m22,m01,m02,m12]
   and q = m00^2+m11^2+m22^2 + 2*(m01^2+m02^2+m12^2), per patch
     v = [m(6), 1, q],  u = [-2*m_diag(3), -4*m_off(3), q, 1]
   give raw[i,j] = u_i . v_j = q_i + q_j - 2*<gram_i, gram_j>, and
   out = raw*s + t with s = 1/(M-m), t = -m*s from one AllReduce(max) of
   [max, -min].

Sharding: patch dim across 8 cores (core k owns image rows [160k,160k+160)
= patches [800k,800k+800)).  Each core builds u/v for its 800 patches,
AllGathers v (8x800 f32), computes its [800,6400] slice of raw twice
(pass 1 reduces min/max, pass 2 applies the affine and writes out).
"""

import numpy as np

import concourse.bass as bass
import concourse.mybir as mybir
import concourse.tile as tile
from concourse.bass_utils import run_bass_kernel_spmd

F32 = mybir.dt.float32
F32R = mybir.dt.float32r
I32 = mybir.dt.int32
AX = mybir.AluOpType
AFT = mybir.ActivationFunctionType

N_CORES = 8
C, H, W = 3, 1280, 1280
PS = 16
HP, WP = H // PS, W // PS            # 80, 80
N = HP * WP                          # 6400
H_LOC = H // N_CORES                 # 160 image rows per core
TP = H_LOC // PS                     # 10 patch-rows per core
N_LOC = TP * WP                      # 800 patches per core
K = 8                                # feature dim of u/v
PP = PS * PS                         # 256 pixels per patch

# feature order: diagonals first, then off-diagonals
PAIRS = [(0, 0), (1, 1), (2, 2), (0, 1), (0, 2), (1, 2)]
NEG_BIG = -3.0e38
CENTER = 256.0

# "f32r" = float32r matmuls (2 cyc/row, inputs rounded to ~TF32 precision)
# "f32"  = plain float32 matmuls (4 cyc/row, exact)
MM_MODE = "f32r"

# output tiling
M_TILES = [(ms, min(128, N_LOC - ms)) for ms in range(0, N_LOC, 128)]   # 7
N_TILES = [(ns, min(512, N - ns)) for ns in range(0, N, 512)]           # 13
NTOT = len(M_TILES) * len(N_TILES)                                      # 91

# walrus in this container accepts at most 1 sync-wait command per
# instruction; Tile's tail drain can carry several.  Split extras onto
# preceding NOPs on the same engine (stream order preserves semantics).
_MAX_WAITS = 1


def _split_sync_waits(nc):
    n_fixed = 0
    for func in nc.m.functions:
        for bb in func.blocks:
            new_insts = []
            for inst in bb.instructions:
                si = inst.sync_info
                if si is not None and si.on_wait and len(si.on_wait) > _MAX_WAITS:
                    waits = list(si.on_wait)
                    keep = waits[-_MAX_WAITS:]
                    extra = waits[:-_MAX_WAITS]
                    chunks = [
                        extra[i : i + _MAX_WAITS]
                        for i in range(0, len(extra), _MAX_WAITS)
                    ]
                    for ci, chunk in enumerate(chunks):
                        nop = mybir.InstNoOp(
                            name=f"{inst.name}-waitsplit-{ci}",
                            engine=inst.engine,
                            ins=[],
                            outs=[],
                            sync_info=mybir.SyncInfo(on_wait=chunk, on_update=[]),
                        )
                        new_insts.append(nop)
                        n_fixed += 1
                    si.on_wait = keep
                new_insts.append(inst)
            bb.instructions[:] = new_insts
    return n_fixed


def _build():
    nc = bass.Bass(num_devices=N_CORES)
    x = nc.dram_tensor("x", [C, H_LOC, W], F32, kind="ExternalInput")
    out = nc.dram_tensor("out", [N_LOC, N], F32, kind="ExternalOutput")
    groups = [list(range(N_CORES))]

    with tile.TileContext(nc, num_cores=N_CORES) as tc:
        with (
            tc.tile_pool(name="dram", bufs=1, space="DRAM") as dpool,
            tc.tile_pool(name="cst", bufs=1) as cst,
            tc.tile_pool(name="prodp", bufs=2) as prodp,
            tc.tile_pool(name="scr", bufs=4) as scrp,
            tc.tile_pool(name="obig", bufs=2) as obig,
            tc.tile_pool(name="ps", bufs=5, space="PSUM") as psp,
        ):
            v_dram = dpool.tile([K, N_LOC], F32, name="v_dram")
            vall = dpool.tile([K * N_CORES, N_LOC], F32, addr_space="Shared",
                              name="vall")
            cc_in = dpool.tile([1, 8], F32, name="cc_in")
            cc_out = dpool.tile([1, 8], F32, addr_space="Shared", name="cc_out")

            # identity matrix for PE transposes
            iota2d = cst.tile([128, 128], F32, name="iota2d")
            nc.gpsimd.iota(iota2d[:, :], pattern=[[1, 128]], base=0,
                           channel_multiplier=0,
                           allow_small_or_imprecise_dtypes=True)
            iota_col = cst.tile([128, 1], F32, name="iota_col")
            nc.gpsimd.iota(iota_col[:, :], pattern=[[0, 1]], base=0,
                           channel_multiplier=1,
                           allow_small_or_imprecise_dtypes=True)
            ident = cst.tile([128, 128], F32, name="ident")
            nc.vector.tensor_scalar(
                out=ident[:, :], in0=iota2d[:, :], scalar1=iota_col[:, 0:1],
                scalar2=None, op0=AX.is_equal,
            )

            # ---- phase A: load all patches, one DMA per channel ----
            # xall[c][w, t*256 + a*16 + b] = x[c, 16t+a, 16w+b]
            xall = []
            dma_engines = [nc.sync, nc.scalar, nc.gpsimd]
            for c in range(C):
                xc = cst.tile([WP, TP * PP], F32, name=f"xall{c}")
                dma_engines[c].dma_start(
                    xc.rearrange("w (t a b) -> w t a b", a=PS, b=PS),
                    x[c].rearrange("(t a) (w b) -> w t a b", a=PS, b=PS),
                )
                xall.append(xc)

            # va/ua hold per-patch features, column layout 8t+slot
            va = cst.tile([WP, K * TP], F32, name="va")
            ua = cst.tile([WP, K * TP], F32, name="ua")
            va_r = va.rearrange("p (t s) -> p t s", s=K)
            ua_r = ua.rearrange("p (t s) -> p t s", s=K)

            # gram features: product then grouped per-t reduce
            for f, (a, b) in enumerate(PAIRS):
                prodb = prodp.tile([WP, TP * PP], F32, name="prodb", tag="prodb")
                if a == b:
                    nc.scalar.activation(prodb[:, :], xall[a][:, :], AFT.Square)
                else:
                    nc.vector.tensor_mul(prodb[:, :], xall[a][:, :], xall[b][:, :])
                nc.vector.tensor_reduce(
                    out=va_r[:, :, f : f + 1],
                    in_=prodb.rearrange("p (t e) -> p t e", e=PP),
                    axis=mybir.AxisListType.X,
                    op=AX.add,
                )
            # center diagonal gram entries (features 0..2)
            nc.vector.tensor_scalar_add(va_r[:, :, 0:3], va_r[:, :, 0:3], -CENTER)
            # q = sum(diag^2) + 2*sum(off^2)
            msq = cst.tile([WP, 6 * TP], F32, name="msq")
            msq_r = msq.rearrange("p (t s) -> p t s", s=6)
            nc.vector.tensor_mul(msq_r[:, :, :], va_r[:, :, 0:6], va_r[:, :, 0:6])
            qd = cst.tile([WP, TP], F32, name="qd")
            qo = cst.tile([WP, TP], F32, name="qo")
            nc.vector.tensor_reduce(
                out=qd[:, :], in_=msq_r[:, :, 0:3],
                axis=mybir.AxisListType.X, op=AX.add,
            )
            nc.vector.tensor_reduce(
                out=qo[:, :], in_=msq_r[:, :, 3:6],
                axis=mybir.AxisListType.X, op=AX.add,
            )
            nc.vector.scalar_tensor_tensor(
                out=va_r[:, :, 7:8].rearrange("p t s -> p (t s)"),
                in0=qo[:, :], scalar=2.0, in1=qd[:, :],
                op0=AX.mult, op1=AX.add,
            )
            nc.vector.memset(va_r[:, :, 6:7], 1.0)
            # u features
            nc.scalar.activation(ua_r[:, :, 0:3], va_r[:, :, 0:3], AFT.Copy,
                                 scale=-2.0)
            nc.scalar.activation(ua_r[:, :, 3:6], va_r[:, :, 3:6], AFT.Copy,
                                 scale=-4.0)
            nc.scalar.activation(ua_r[:, :, 6:7], va_r[:, :, 7:8], AFT.Copy)
            nc.vector.memset(ua_r[:, :, 7:8], 1.0)

            # transpose [80, 80] feature blocks -> [8, 800] operand layouts
            v_sbT = cst.tile([K, N_LOC], F32, name="v_sbT")
            lhsT = cst.tile([K, N_LOC], F32, name="lhsT")
            for src_r, dst in ((va_r, v_sbT), (ua_r, lhsT)):
                for t in range(TP):
                    ps_tr = psp.tile([K, WP], F32, name="ps_tr", tag="ps_tr",
                                     bufs=2)
                    nc.tensor.transpose(
                        ps_tr[0:K, 0:WP],
                        src_r[:, t, :],
                        ident[0:WP, 0:WP],
                    )
                    nc.vector.tensor_copy(
                        dst[:, WP * t : WP * (t + 1)], ps_tr[0:K, 0:WP]
                    )
            nc.sync.dma_start(v_dram[:, :], v_sbT[:, :])

            # ---- all-gather v across cores ----
            nc.gpsimd.collective_compute(
                "AllGather",
                AX.bypass,
                replica_groups=groups,
                ins=[v_dram.opt()],
                outs=[vall.opt()],
            )

            rhs = cst.tile([K, N], F32, name="rhs")
            nc.sync.dma_start(
                rhs.rearrange("f (r l) -> f r l", l=N_LOC),
                vall.rearrange("(r f) l -> f r l", f=K),
            )

            if MM_MODE == "f32r":
                mm_rhs = cst.tile([K, N], F32R, name="mm_rhs")
                nc.vector.tensor_copy(mm_rhs[:, :], rhs[:, :])
                mm_lhsT = cst.tile([K, N_LOC], F32R, name="mm_lhsT")
                nc.vector.tensor_copy(mm_lhsT[:, :], lhsT[:, :])
            else:
                mm_rhs, mm_lhsT = rhs, lhsT

            # ---- pass 1: min/max of raw ----
            racc = cst.tile([128, 2 * NTOT], F32, name="racc")
            nc.vector.memset(racc[:, :], NEG_BIG)
            idx = 0
            for ms, mh in M_TILES:
                for ns, nw in N_TILES:
                    ps_t = psp.tile([128, 512], F32, name="ps1", tag="ps")
                    nc.tensor.matmul(
                        ps_t[0:mh, 0:nw],
                        lhsT=mm_lhsT[:, ms : ms + mh],
                        rhs=mm_rhs[:, ns : ns + nw],
                        start=True,
                        stop=True,
                    )
                    sc = scrp.tile([128, 512], F32, name="sc", tag="sc")
                    nc.scalar.activation(sc[0:mh, 0:nw], ps_t[0:mh, 0:nw], AFT.Copy)
                    nc.vector.tensor_reduce(
                        out=racc[0:mh, idx : idx + 1],
                        in_=sc[0:mh, 0:nw],
                        axis=mybir.AxisListType.X,
                        op=AX.max,
                    )
                    nc.vector.tensor_reduce(
                        out=racc[0:mh, NTOT + idx : NTOT + idx + 1],
                        in_=sc[0:mh, 0:nw],
                        axis=mybir.AxisListType.X,
                        op=AX.min,
                        negate=True,
                    )
                    idx += 1

            # fold -> [128,2] = [max, -min]; partitions via DRAM transpose; cores
            gpair = cst.tile([128, 2], F32, name="gpair")
            nc.vector.tensor_reduce(
                out=gpair[:, :],
                in_=racc.rearrange("p (h n) -> p h n", h=2),
                axis=mybir.AxisListType.X,
                op=AX.max,
            )
            gp_dram = dpool.tile([128, 2], F32, name="gp_dram")
            nc.sync.dma_start(gp_dram[:, :], gpair[:, :])
            gpT = cst.tile([2, 128], F32, name="gpT")
            nc.sync.dma_start(gpT[:, :], gp_dram.rearrange("w f -> f w"))
            gg = cst.tile([2, 1], F32, name="gg")
            nc.vector.tensor_reduce(
                out=gg[:, :], in_=gpT[:, :], axis=mybir.AxisListType.X, op=AX.max
            )
            sc8 = cst.tile([1, 8], F32, name="sc8")
            nc.vector.memset(sc8[:, :], NEG_BIG)
            nc.sync.dma_start(cc_in[:, :], sc8[:, :])
            nc.sync.dma_start(cc_in[0:1, 0:2].rearrange("a b -> b a"), gg[:, :])
            nc.gpsimd.collective_compute(
                "AllReduce",
                AX.max,
                replica_groups=groups,
                ins=[cc_in.opt()],
                outs=[cc_out.opt()],
            )
            mm_sb = cst.tile([1, 2], F32, name="mm_sb")
            nc.sync.dma_start(mm_sb[:, :], cc_out[0:1, 0:2])
            # broadcast [1,2] -> [128,2] via ones-matmul into PSUM
            ones_col = cst.tile([1, 128], F32, name="ones_col")
            nc.vector.memset(ones_col[:, :], 1.0)
            ps_bc = psp.tile([128, 2], F32, name="ps_bc", tag="ps_bc", bufs=1)
            nc.tensor.matmul(
                ps_bc[:, :],
                lhsT=ones_col[0:1, :],
                rhs=mm_sb[0:1, :],
                start=True,
                stop=True,
            )
            # s = 1/(M - m); t = -m * s   (bc_sb = [M, -m] per partition)
            bc_sb = cst.tile([128, 2], F32, name="bc_sb")
            nc.vector.tensor_copy(bc_sb[:, :], ps_bc[:, :])
            rng = cst.tile([128, 1], F32, name="rng")
            nc.vector.tensor_add(rng[:, :], bc_sb[:, 0:1], bc_sb[:, 1:2])
            s_sb = cst.tile([128, 1], F32, name="s_sb")
            nc.vector.reciprocal(s_sb[:, :], rng[:, :])
            t_sb = cst.tile([128, 1], F32, name="t_sb")
            nc.vector.tensor_mul(t_sb[:, :], bc_sb[:, 1:2], s_sb[:, :])

            # ---- pass 2: recompute, normalize on ACT, write out in row blocks
            for ms, mh in M_TILES:
                ob = obig.tile([128, N], F32, name="ob", tag="ob")
                for ns, nw in N_TILES:
                    ps_t = psp.tile([128, 512], F32, name="ps2", tag="ps")
                    nc.tensor.matmul(
                        ps_t[0:mh, 0:nw],
                        lhsT=mm_lhsT[:, ms : ms + mh],
                        rhs=mm_rhs[:, ns : ns + nw],
                        start=True,
                        stop=True,
                    )
                    nc.scalar.activation(
                        ob[0:mh, ns : ns + nw],
                        ps_t[0:mh, 0:nw],
                        AFT.Identity,
                        bias=t_sb[0:mh, 0:1],
                        scale=s_sb[0:mh, 0:1],
                    )
                nc.sync.dma_start(out[ms : ms + mh, :], ob[0:mh, :])

    _split_sync_waits(nc)
    return nc


_NC_CACHE = []


def kernel(batch: np.ndarray) -> np.ndarray:
    batch = np.asarray(batch, dtype=np.float32)
    assert batch.shape == (C, H, W)
    if not _NC_CACHE:
        _NC_CACHE.append(_build())
    nc = _NC_CACHE[0]
    in_maps = [
        {"x": np.ascontiguousarray(batch[:, k * H_LOC : (k + 1) * H_LOC, :])}
        for k in range(N_CORES)
    ]
    res = run_bass_kernel_spmd(nc, in_maps, core_ids=list(range(N_CORES)))
    full = np.concatenate([res.results[k]["out"] for k in range(N_CORES)], axis=0)
    return full[None].astype(np.float32)


# revision 24
# speedup vs baseline: 1.5000x; 1.0894x over previous
"""Distributed Trainium2 kernel for nn_Curating_of_attention_mask.

Math: batch (3,1280,1280) -> 6400 patches of 16x16 -> per-patch 3x3 channel
gram -> pairwise squared-distance matrix (6400,6400) -> global min/max
normalize -> (1,6400,6400).

Key simplifications:
 - (d - min)/(max - min) is invariant to positive affine rescaling of d, so
   the reference's /768 and /9 factors are dropped.
 - Distances are invariant to subtracting a constant vector from every
   patch's gram, so grams are centered by the expected gram of unit-variance
   noise (256 on the diagonal); this shrinks magnitudes ~30x and with it
   every rounding error.
 - With the 6 unique (centered) gram entries m = [m00,m11,m22,m01,m02,m12]
   and q = m00^2+m11^2+m22^2 + 2*(m01^2+m02^2+m12^2), per patch
     v = [m(6), 1, q],  u = [-2*m_diag(3), -4*m_off(3), q, 1]
   give raw[i,j] = u_i . v_j = q_i + q_j - 2*<gram_i, gram_j>, and
   out = raw*s + t with s = 1/(M-m), t = -m*s from one AllReduce(max) of
   [max, -min].

Sharding: patch dim across 8 cores (core k owns image rows [160k,160k+160)
= patches [800k,800k+800)).  Each core builds u/v for its 800 patches,
AllGathers v (8x800 f32), computes its [800,6400] slice of raw twice
(pass 1 reduces min/max, pass 2 applies the affine and writes out).
"""

import numpy as np

import concourse.bass as bass
import concourse.mybir as mybir
import concourse.tile as tile
from concourse.bass_utils import run_bass_kernel_spmd

F32 = mybir.dt.float32
F32R = mybir.dt.float32r
I32 = mybir.dt.int32
AX = mybir.AluOpType
AFT = mybir.ActivationFunctionType

N_CORES = 8
C, H, W = 3, 1280, 1280
PS = 16
HP, WP = H // PS, W // PS            # 80, 80
N = HP * WP                          # 6400
H_LOC = H // N_CORES                 # 160 image rows per core
TP = H_LOC // PS                     # 10 patch-rows per core
N_LOC = TP * WP                      # 800 patches per core
K = 8                                # feature dim of u/v
PP = PS * PS                         # 256 pixels per patch

# feature order: diagonals first, then off-diagonals
PAIRS = [(0, 0), (1, 1), (2, 2), (0, 1), (0, 2), (1, 2)]
NEG_BIG = -3.0e38
CENTER = 256.0

# "f32r" = float32r matmuls (2 cyc/row, inputs rounded to ~TF32 precision)
# "f32"  = plain float32 matmuls (4 cyc/row, exact)
MM_MODE = "f32r"

# output tiling
M_TILES = [(ms, min(128, N_LOC - ms)) for ms in range(0, N_LOC, 128)]   # 7
N_TILES = [(ns, min(512, N - ns)) for ns in range(0, N, 512)]           # 13
NTOT = len(M_TILES) * len(N_TILES)                                      # 91

# walrus in this container accepts at most 1 sync-wait command per
# instruction; Tile's tail drain can carry several.  Split extras onto
# preceding NOPs on the same engine (stream order preserves semantics).
_MAX_WAITS = 1


def _split_sync_waits(nc):
    n_fixed = 0
    for func in nc.m.functions:
        for bb in func.blocks:
            new_insts = []
            for inst in bb.instructions:
                si = inst.sync_info
                if si is not None and si.on_wait and len(si.on_wait) > _MAX_WAITS:
                    waits = list(si.on_wait)
                    keep = waits[-_MAX_WAITS:]
                    extra = waits[:-_MAX_WAITS]
                    chunks = [
                        extra[i : i + _MAX_WAITS]
                        for i in range(0, len(extra), _MAX_WAITS)
                    ]
                    for ci, chunk in enumerate(chunks):
                        nop = mybir.InstNoOp(
                            name=f"{inst.name}-waitsplit-{ci}",
                            engine=inst.engine,
                            ins=[],
                            outs=[],
                            sync_info=mybir.SyncInfo(on_wait=chunk, on_update=[]),
                        )
                        new_insts.append(nop)
                        n_fixed += 1
                    si.on_wait = keep
                new_insts.append(inst)
            bb.instructions[:] = new_insts
    return n_fixed


def _build():
    nc = bass.Bass(num_devices=N_CORES)
    x = nc.dram_tensor("x", [C, H_LOC, W], F32, kind="ExternalInput")
    out = nc.dram_tensor("out", [N_LOC, N], F32, kind="ExternalOutput")
    groups = [list(range(N_CORES))]

    with tile.TileContext(nc, num_cores=N_CORES) as tc:
        with (
            tc.tile_pool(name="dram", bufs=1, space="DRAM") as dpool,
            tc.tile_pool(name="cst", bufs=1) as cst,
            tc.tile_pool(name="scr", bufs=2) as scrp,
            tc.tile_pool(name="obig", bufs=2) as obig,
            tc.tile_pool(name="ps", bufs=5, space="PSUM") as psp,
        ):
            v_dram = dpool.tile([K, N_LOC], F32, name="v_dram")
            vall = dpool.tile([K * N_CORES, N_LOC], F32, addr_space="Shared",
                              name="vall")
            cc_in = dpool.tile([1, 8], F32, name="cc_in")
            cc_out = dpool.tile([1, 8], F32, addr_space="Shared", name="cc_out")

            # identity matrix for PE transposes
            iota2d = cst.tile([128, 128], F32, name="iota2d")
            nc.gpsimd.iota(iota2d[:, :], pattern=[[1, 128]], base=0,
                           channel_multiplier=0,
                           allow_small_or_imprecise_dtypes=True)
            iota_col = cst.tile([128, 1], F32, name="iota_col")
            nc.gpsimd.iota(iota_col[:, :], pattern=[[0, 1]], base=0,
                           channel_multiplier=1,
                           allow_small_or_imprecise_dtypes=True)
            ident = cst.tile([128, 128], F32, name="ident")
            nc.vector.tensor_scalar(
                out=ident[:, :], in0=iota2d[:, :], scalar1=iota_col[:, 0:1],
                scalar2=None, op0=AX.is_equal,
            )

            # va/ua hold per-patch features, column layout 8t+slot
            va = cst.tile([WP, K * TP], F32, name="va")
            ua = cst.tile([WP, K * TP], F32, name="ua")
            va_r = va.rearrange("p (t s) -> p t s", s=K)
            ua_r = ua.rearrange("p (t s) -> p t s", s=K)

            # ---- phase A: load patches in 2 half-slabs per channel so gram
            # compute overlaps the loads; pools close afterwards to free SBUF
            HT = TP // 2  # patch-rows per half
            with (
                tc.tile_pool(name="phA", bufs=1) as phap,
                tc.tile_pool(name="prodp", bufs=2) as prodp,
            ):
                xall = []
                dma_engines = [nc.sync, nc.scalar, nc.gpsimd]
                for c in range(C):
                    xc = phap.tile([WP, TP * PP], F32, name=f"xall{c}")
                    xc_r = xc.rearrange("w (t a b) -> w t a b", a=PS, b=PS)
                    for h in range(2):
                        dma_engines[c].dma_start(
                            xc_r[:, HT * h : HT * (h + 1), :, :],
                            x[
                                c, PS * HT * h : PS * HT * (h + 1), :
                            ].rearrange("(t a) (w b) -> w t a b", a=PS, b=PS),
                        )
                    xall.append(xc)

                # gram features: product then grouped per-t reduce, per half
                for h in range(2):
                    tsl = slice(HT * h, HT * (h + 1))
                    csl = slice(HT * PP * h, HT * PP * (h + 1))
                    for f, (a, b) in enumerate(PAIRS):
                        prodb = prodp.tile([WP, HT * PP], F32, name="prodb",
                                           tag="prodb")
                        if a == b:
                            nc.scalar.activation(
                                prodb[:, :], xall[a][:, csl], AFT.Square
                            )
                        else:
                            nc.vector.tensor_mul(
                                prodb[:, :], xall[a][:, csl], xall[b][:, csl]
                            )
                        nc.vector.tensor_reduce(
                            out=va_r[:, tsl, f : f + 1],
                            in_=prodb.rearrange("p (t e) -> p t e", e=PP),
                            axis=mybir.AxisListType.X,
                            op=AX.add,
                        )
            # center diagonal gram entries (features 0..2)
            nc.vector.tensor_scalar_add(va_r[:, :, 0:3], va_r[:, :, 0:3], -CENTER)
            # q = sum(diag^2) + 2*sum(off^2)
            msq = cst.tile([WP, 6 * TP], F32, name="msq")
            msq_r = msq.rearrange("p (t s) -> p t s", s=6)
            nc.vector.tensor_mul(msq_r[:, :, :], va_r[:, :, 0:6], va_r[:, :, 0:6])
            qd = cst.tile([WP, TP], F32, name="qd")
            qo = cst.tile([WP, TP], F32, name="qo")
            nc.vector.tensor_reduce(
                out=qd[:, :], in_=msq_r[:, :, 0:3],
                axis=mybir.AxisListType.X, op=AX.add,
            )
            nc.vector.tensor_reduce(
                out=qo[:, :], in_=msq_r[:, :, 3:6],
                axis=mybir.AxisListType.X, op=AX.add,
            )
            nc.vector.scalar_tensor_tensor(
                out=va_r[:, :, 7:8].rearrange("p t s -> p (t s)"),
                in0=qo[:, :], scalar=2.0, in1=qd[:, :],
                op0=AX.mult, op1=AX.add,
            )
            nc.vector.memset(va_r[:, :, 6:7], 1.0)
            # u features
            nc.scalar.activation(ua_r[:, :, 0:3], va_r[:, :, 0:3], AFT.Copy,
                                 scale=-2.0)
            nc.scalar.activation(ua_r[:, :, 3:6], va_r[:, :, 3:6], AFT.Copy,
                                 scale=-4.0)
            nc.scalar.activation(ua_r[:, :, 6:7], va_r[:, :, 7:8], AFT.Copy)
            nc.vector.memset(ua_r[:, :, 7:8], 1.0)

            # transpose [80, 80] feature blocks -> [8, 800] operand layouts
            v_sbT = cst.tile([K, N_LOC], F32, name="v_sbT")
            lhsT = cst.tile([K, N_LOC], F32, name="lhsT")
            for src_r, dst in ((va_r, v_sbT), (ua_r, lhsT)):
                for t in range(TP):
                    ps_tr = psp.tile([K, WP], F32, name="ps_tr", tag="ps_tr",
                                     bufs=2)
                    nc.tensor.transpose(
                        ps_tr[0:K, 0:WP],
                        src_r[:, t, :],
                        ident[0:WP, 0:WP],
                    )
                    nc.vector.tensor_copy(
                        dst[:, WP * t : WP * (t + 1)], ps_tr[0:K, 0:WP]
                    )
            nc.sync.dma_start(v_dram[:, :], v_sbT[:, :])

            # ---- all-gather v across cores ----
            nc.gpsimd.collective_compute(
                "AllGather",
                AX.bypass,
                replica_groups=groups,
                ins=[v_dram.opt()],
                outs=[vall.opt()],
            )

            rhs = cst.tile([K, N], F32, name="rhs")
            nc.sync.dma_start(
                rhs.rearrange("f (r l) -> f r l", l=N_LOC),
                vall.rearrange("(r f) l -> f r l", f=K),
            )

            if MM_MODE == "f32r":
                mm_rhs = cst.tile([K, N], F32R, name="mm_rhs")
                nc.vector.tensor_copy(mm_rhs[:, :], rhs[:, :])
                mm_lhsT = cst.tile([K, N_LOC], F32R, name="mm_lhsT")
                nc.vector.tensor_copy(mm_lhsT[:, :], lhsT[:, :])
            else:
                mm_rhs, mm_lhsT = rhs, lhsT

            # ---- pass 1: min/max of raw ----
            # one [128, 6400] staging row-block per M-tile; per-block flat
            # max on GpSimd (otherwise idle) and flat negated-min on DVE
            MPAD = 8
            racc = cst.tile([128, 2 * MPAD], F32, name="racc")
            nc.vector.memset(racc[:, :], NEG_BIG)
            for i, (ms, mh) in enumerate(M_TILES):
                scb = scrp.tile([128, N], F32, name="scb", tag="scb")
                for ns, nw in N_TILES:
                    ps_t = psp.tile([128, 512], F32, name="ps1", tag="ps")
                    nc.tensor.matmul(
                        ps_t[0:mh, 0:nw],
                        lhsT=mm_lhsT[:, ms : ms + mh],
                        rhs=mm_rhs[:, ns : ns + nw],
                        start=True,
                        stop=True,
                    )
                    nc.scalar.activation(
                        scb[0:mh, ns : ns + nw], ps_t[0:mh, 0:nw], AFT.Copy
                    )
                nc.vector.tensor_reduce(
                    out=racc[0:mh, i : i + 1],
                    in_=scb[0:mh, :],
                    axis=mybir.AxisListType.X,
                    op=AX.max,
                )
                nc.vector.tensor_reduce(
                    out=racc[0:mh, MPAD + i : MPAD + i + 1],
                    in_=scb[0:mh, :],
                    axis=mybir.AxisListType.X,
                    op=AX.min,
                    negate=True,
                )

            # fold -> [128,2] = [max, -min]; partitions via DRAM transpose; cores
            gpair = cst.tile([128, 2], F32, name="gpair")
            nc.vector.tensor_reduce(
                out=gpair[:, :],
                in_=racc.rearrange("p (h n) -> p h n", h=2),
                axis=mybir.AxisListType.X,
                op=AX.max,
            )
            gp_dram = dpool.tile([128, 2], F32, name="gp_dram")
            nc.sync.dma_start(gp_dram[:, :], gpair[:, :])
            gpT = cst.tile([2, 128], F32, name="gpT")
            nc.sync.dma_start(gpT[:, :], gp_dram.rearrange("w f -> f w"))
            gg = cst.tile([2, 1], F32, name="gg")
            nc.vector.tensor_reduce(
                out=gg[:, :], in_=gpT[:, :], axis=mybir.AxisListType.X, op=AX.max
            )
            sc8 = cst.tile([1, 8], F32, name="sc8")
            nc.vector.memset(sc8[:, :], NEG_BIG)
            nc.sync.dma_start(cc_in[:, :], sc8[:, :])
            nc.sync.dma_start(cc_in[0:1, 0:2].rearrange("a b -> b a"), gg[:, :])
            nc.gpsimd.collective_compute(
                "AllReduce",
                AX.max,
                replica_groups=groups,
                ins=[cc_in.opt()],
                outs=[cc_out.opt()],
            )
            mm_sb = cst.tile([1, 2], F32, name="mm_sb")
            nc.sync.dma_start(mm_sb[:, :], cc_out[0:1, 0:2])
            # broadcast [1,2] -> [128,2] via ones-matmul into PSUM
            ones_col = cst.tile([1, 128], F32, name="ones_col")
            nc.vector.memset(ones_col[:, :], 1.0)
            ps_bc = psp.tile([128, 2], F32, name="ps_bc", tag="ps_bc", bufs=1)
            nc.tensor.matmul(
                ps_bc[:, :],
                lhsT=ones_col[0:1, :],
                rhs=mm_sb[0:1, :],
                start=True,
                stop=True,
            )
            # s = 1/(M - m); t = -m * s   (bc_sb = [M, -m] per partition)
            bc_sb = cst.tile([128, 2], F32, name="bc_sb")
            nc.vector.tensor_copy(bc_sb[:, :], ps_bc[:, :])
            rng = cst.tile([128, 1], F32, name="rng")
            nc.vector.tensor_add(rng[:, :], bc_sb[:, 0:1], bc_sb[:, 1:2])
            s_sb = cst.tile([128, 1], F32, name="s_sb")
            nc.vector.reciprocal(s_sb[:, :], rng[:, :])
            t_sb = cst.tile([128, 1], F32, name="t_sb")
            nc.vector.tensor_mul(t_sb[:, :], bc_sb[:, 1:2], s_sb[:, :])

            # ---- pass 2: recompute, normalize (ACT/DVE alternating), write
            # out in full row blocks
            k2 = 0
            for ms, mh in M_TILES:
                ob = obig.tile([128, N], F32, name="ob", tag="ob")
                for ns, nw in N_TILES:
                    ps_t = psp.tile([128, 512], F32, name="ps2", tag="ps")
                    nc.tensor.matmul(
                        ps_t[0:mh, 0:nw],
                        lhsT=mm_lhsT[:, ms : ms + mh],
                        rhs=mm_rhs[:, ns : ns + nw],
                        start=True,
                        stop=True,
                    )
                    if k2 % 2 == 0:
                        nc.scalar.activation(
                            ob[0:mh, ns : ns + nw],
                            ps_t[0:mh, 0:nw],
                            AFT.Identity,
                            bias=t_sb[0:mh, 0:1],
                            scale=s_sb[0:mh, 0:1],
                        )
                    else:
                        nc.vector.tensor_scalar(
                            out=ob[0:mh, ns : ns + nw],
                            in0=ps_t[0:mh, 0:nw],
                            scalar1=s_sb[0:mh, 0:1],
                            scalar2=t_sb[0:mh, 0:1],
                            op0=AX.mult,
                            op1=AX.add,
                        )
                    k2 += 1
                nc.sync.dma_start(out[ms : ms + mh, :], ob[0:mh, :])

    _split_sync_waits(nc)
    return nc


_NC_CACHE = []


def kernel(batch: np.ndarray) -> np.ndarray:
    batch = np.asarray(batch, dtype=np.float32)
    assert batch.shape == (C, H, W)
    if not _NC_CACHE:
        _NC_CACHE.append(_build())
    nc = _NC_CACHE[0]
    in_maps = [
        {"x": np.ascontiguousarray(batch[:, k * H_LOC : (k + 1) * H_LOC, :])}
        for k in range(N_CORES)
    ]
    res = run_bass_kernel_spmd(nc, in_maps, core_ids=list(range(N_CORES)))
    full = np.concatenate([res.results[k]["out"] for k in range(N_CORES)], axis=0)
    return full[None].astype(np.float32)
